# revision 5
# baseline (speedup 1.0000x reference)
"""Trainium2 Bass kernel for the Cortex top-k masking module.

Computes, for inputs x_input/x_context [B,D], per-column weights
W_ff/W_ctx [C,D,N], W_pred [C,N,D], bias/avg_activity [C,N], k:

    drive   = x@W_ff + 0.3*(x_ctx@W_ctx) + bias          [B,C,N]
    boosted = drive + log1p(0.05/(avg+1e-6))             (homeostasis*100 = 1.0)
    mask    = boosted >= (k-th largest of boosted per row)
    act     = relu(drive)*mask;  act = act/(act.sum(-1)+1e-8)*k
    pred    = act @ W_pred                                [B,C,D]
    err     = x_input - pred

Sharding: columns C split across 8 NeuronCores (8 each), x replicated.
"""

import os
import numpy as np

P = 128
B, C, D, N = 1024, 64, 256, 512
NCORES = 8
C_LOC = C // NCORES   # 8 columns per core
BT = B // P           # 8 b-tiles per core
KD = D // P           # 2 contraction chunks for the drive matmuls
KN = N // P           # 4 contraction chunks for the prediction matmul

_NEG_FILL = -3.0e38   # match_replace fill; far below any real boosted value

_cache = {}


def ts(i, size):
    return slice(i * size, (i + 1) * size)


def _build(k):
    """Trace + compile the per-core kernel. Returns the compiled Bass module."""
    import concourse.bass as bass  # noqa: F401
    import concourse.bacc as bacc
    import concourse.mybir as mybir
    from concourse import tile
    from concourse.masks import make_identity

    f32 = mybir.dt.float32
    Alu = mybir.AluOpType
    Act = mybir.ActivationFunctionType

    k = int(k)
    assert 1 <= k <= N
    rounds = (k + 7) // 8          # max8 rounds needed to reach the k-th value
    last_idx = (k - 1) % 8

    nc = bacc.Bacc("TRN2", target_bir_lowering=False, debug=False,
                   num_devices=NCORES)

    xT_d = nc.declare_dram_parameter("xT", [D, B], f32, isOutput=False)
    xcT_d = nc.declare_dram_parameter("xcT", [D, B], f32, isOutput=False)
    x_d = nc.declare_dram_parameter("x", [B, D], f32, isOutput=False)
    wff_d = nc.declare_dram_parameter("wff", [C_LOC, D, N], f32, isOutput=False)
    wctx_d = nc.declare_dram_parameter("wctx", [C_LOC, D, N], f32, isOutput=False)
    wp_d = nc.declare_dram_parameter("wp", [C_LOC, N, D], f32, isOutput=False)
    btb_d = nc.declare_dram_parameter("btb", [C_LOC, N], f32, isOutput=False)
    bt_d = nc.declare_dram_parameter("bt", [C_LOC, N], f32, isOutput=False)

    act_d = nc.declare_dram_parameter("act", [B, C_LOC, N], f32, isOutput=True)
    pred_d = nc.declare_dram_parameter("pred", [B, C_LOC, D], f32, isOutput=True)
    err_d = nc.declare_dram_parameter("err", [B, C_LOC, D], f32, isOutput=True)

    with tile.TileContext(nc) as tc:
        with (
            tc.tile_pool(name="const", bufs=1) as const,
            tc.tile_pool(name="wpool", bufs=2) as wpool,
            tc.tile_pool(name="work", bufs=3) as work,
            tc.tile_pool(name="psA", bufs=2, space="PSUM") as psA,
            tc.tile_pool(name="psT", bufs=2, space="PSUM") as psT,
            tc.tile_pool(name="psP", bufs=2, space="PSUM") as psP,
            tc.tile_pool(name="psB", bufs=1, space="PSUM") as psB,
        ):
            # ---- resident constants ----
            ident = const.tile([P, P], f32)
            make_identity(nc, ident[:])

            ones_row = const.tile([1, P], f32)
            nc.vector.memset(ones_row[:], 1.0)

            # rows kept on partition 0 so K=1 matmul rhs slices are legal
            btb_s = const.tile([1, C_LOC, N], f32)   # bias + boost rows
            nc.sync.dma_start(btb_s[:], btb_d[:].rearrange("(o c) n -> o c n", o=1))
            bt_s = const.tile([1, C_LOC, N], f32)    # boost-only rows
            nc.sync.dma_start(bt_s[:], bt_d[:].rearrange("(o c) n -> o c n", o=1))

            xT_s = const.tile([P, KD, B], f32)
            nc.sync.dma_start(xT_s[:], xT_d[:].rearrange("(kc p) b -> p kc b", p=P))
            xcT_s = const.tile([P, KD, B], f32)
            nc.sync.dma_start(xcT_s[:], xcT_d[:].rearrange("(kc p) b -> p kc b", p=P))
            x_s = const.tile([P, BT, D], f32)
            nc.sync.dma_start(x_s[:], x_d[:].rearrange("(bt p) d -> p bt d", p=P))

            # bt broadcast tiles [128, N] per local column, built once via a
            # K=1 outer-product matmul (ones_col x bt_row) + ACT copy-out.
            bt_bc = const.tile([P, C_LOC, N], f32)
            for ci in range(C_LOC):
                ps = psB.tile([P, N], f32)
                nc.tensor.matmul(ps[:], lhsT=ones_row[:], rhs=bt_s[:, ci, :],
                                 start=True, stop=True)
                nc.scalar.activation(bt_bc[:, ci, :], ps[:], Act.Copy)

            inv_k = float(1.0 / k)

            for ci in range(C_LOC):
                # ---- per-column weights ----
                wff_t = wpool.tile([P, KD, N], f32)
                nc.sync.dma_start(
                    wff_t[:], wff_d[ci].rearrange("(kc p) n -> p kc n", p=P))
                wctx_t = wpool.tile([P, KD, N], f32)
                nc.sync.dma_start(
                    wctx_t[:], wctx_d[ci].rearrange("(kc p) n -> p kc n", p=P))
                wp_t = wpool.tile([P, KN, D], f32)
                nc.sync.dma_start(
                    wp_t[:], wp_d[ci].rearrange("(kc p) d -> p kc d", p=P))

                for bi in range(BT):
                    # ---- boosted = x@Wff + 0.3*xc@Wctx + (bias+boost) ----
                    pA = psA.tile([P, N], f32)
                    nc.tensor.matmul(pA[:], lhsT=xT_s[:, 0, ts(bi, P)],
                                     rhs=wff_t[:, 0, :], start=True, stop=False)
                    for kc in range(1, KD):
                        nc.tensor.matmul(pA[:], lhsT=xT_s[:, kc, ts(bi, P)],
                                         rhs=wff_t[:, kc, :], start=False, stop=False)
                    for kc in range(KD):
                        nc.tensor.matmul(pA[:], lhsT=xcT_s[:, kc, ts(bi, P)],
                                         rhs=wctx_t[:, kc, :], start=False, stop=False)
                    nc.tensor.matmul(pA[:], lhsT=ones_row[:],
                                     rhs=btb_s[:, ci, :], start=False, stop=True)

                    boosted = work.tile([P, N], f32)
                    nc.scalar.activation(boosted[:], pA[:], Act.Copy)

                    # drive = boosted - boost  (recovers raw + bias)
                    drive = work.tile([P, N], f32)
                    nc.gpsimd.tensor_sub(drive[:], boosted[:], bt_bc[:, ci, :])

                    # ---- top-k threshold via 8-wide max + match_replace ----
                    m = work.tile([P, 8], f32, tag="m8")
                    nc.vector.max(m[:], boosted[:])
                    src = boosted
                    for r in range(1, rounds):
                        nxt = work.tile([P, N], f32, tag=f"mr{r}")
                        nc.vector.match_replace(nxt[:], m[:], src[:], _NEG_FILL)
                        m = work.tile([P, 8], f32, tag="m8")
                        nc.vector.max(m[:], nxt[:])
                        src = nxt
                    thr = m[:, last_idx:last_idx + 1]

                    # dm = (boosted >= thr) * drive   (one fused DVE op)
                    dm = work.tile([P, N], f32)
                    nc.vector.scalar_tensor_tensor(
                        dm[:], boosted[:], thr, drive[:],
                        op0=Alu.is_ge, op1=Alu.mult)

                    # masked = relu(dm), rowsum accumulated for free
                    masked = work.tile([P, N], f32)
                    rowsum = work.tile([P, 1], f32)
                    nc.scalar.activation(masked[:], dm[:], Act.Relu,
                                         accum_out=rowsum[:])

                    # s = k / (rowsum + 1e-8)
                    t1 = work.tile([P, 1], f32)
                    nc.vector.tensor_scalar(t1[:], rowsum[:], 1e-8, inv_k,
                                            op0=Alu.add, op1=Alu.mult)
                    s = work.tile([P, 1], f32)
                    nc.vector.reciprocal(s[:], t1[:])

                    # final activation out = masked * s
                    act_out = work.tile([P, N], f32)
                    nc.gpsimd.tensor_scalar_mul(act_out[:], masked[:], s[:])
                    nc.sync.dma_start(act_d[ts(bi, P), ci, :], act_out[:])

                    # ---- transpose masked -> actT for the pred matmul ----
                    pT = psT.tile([P, N], f32)
                    for j in range(KN):
                        nc.tensor.transpose(pT[:, ts(j, P)], masked[:, ts(j, P)],
                                            ident[:])
                    actT = work.tile([P, N], f32)
                    nc.scalar.activation(actT[:], pT[:], Act.Copy)

                    # pred = (masked @ Wp) * s
                    pP = psP.tile([P, D], f32)
                    for j in range(KN):
                        nc.tensor.matmul(pP[:], lhsT=actT[:, ts(j, P)],
                                         rhs=wp_t[:, j, :],
                                         start=(j == 0), stop=(j == KN - 1))
                    pred_s = work.tile([P, D], f32)
                    nc.scalar.activation(pred_s[:], pP[:], Act.Copy, scale=s[:])
                    nc.sync.dma_start(pred_d[ts(bi, P), ci, :], pred_s[:])

                    # err = x - pred
                    err_s = work.tile([P, D], f32)
                    nc.gpsimd.tensor_sub(err_s[:], x_s[:, bi, :], pred_s[:])
                    nc.sync.dma_start(err_d[ts(bi, P), ci, :], err_s[:])

    nc.compile()
    return nc


def _get_nc(k):
    k = int(k)
    if k not in _cache:
        _cache[k] = _build(k)
    return _cache[k]


def _prep_inputs(x_input, x_context, W_ff, W_ctx, W_pred, bias, avg_activity):
    """Host-side shard prep: transposes, feedback scale, boost row."""
    x_input = np.ascontiguousarray(x_input, dtype=np.float32)
    xT = np.ascontiguousarray(x_input.T)
    xcT = np.ascontiguousarray((np.asarray(x_context, np.float32) * np.float32(0.3)).T)
    boost = np.log1p(np.float32(0.05) / (np.asarray(avg_activity, np.float32)
                                         + np.float32(1e-6))).astype(np.float32)
    btb = (np.asarray(bias, np.float32) + boost).astype(np.float32)

    in_maps = []
    for core in range(NCORES):
        cs = ts(core, C_LOC)
        in_maps.append({
            "xT": xT,
            "xcT": xcT,
            "x": x_input,
            "wff": np.ascontiguousarray(W_ff[cs], np.float32),
            "wctx": np.ascontiguousarray(W_ctx[cs], np.float32),
            "wp": np.ascontiguousarray(W_pred[cs], np.float32),
            "btb": np.ascontiguousarray(btb[cs]),
            "bt": np.ascontiguousarray(boost[cs]),
        })
    return in_maps


def _run(inputs, trace=False):
    from concourse.bass_utils import run_bass_kernel_spmd

    k = int(inputs["k"])
    nc = _get_nc(k)
    in_maps = _prep_inputs(
        inputs["x_input"], inputs["x_context"], inputs["W_ff"],
        inputs["W_ctx"], inputs["W_pred"], inputs["bias"],
        inputs["avg_activity"])
    res = run_bass_kernel_spmd(nc, in_maps, list(range(NCORES)), trace=trace)
    act = np.concatenate([r["act"] for r in res.results], axis=1)
    pred = np.concatenate([r["pred"] for r in res.results], axis=1)
    err = np.concatenate([r["err"] for r in res.results], axis=1)
    return (act, pred, err), res


def kernel(**inputs):
    outs, _ = _run(inputs, trace=False)
    return outs


# revision 15
# speedup vs baseline: 1.8551x; 1.8551x over previous
"""Trainium2 Bass kernel for the Cortex top-k masking module.

Computes, for inputs x_input/x_context [B,D], per-column weights
W_ff/W_ctx [C,D,N], W_pred [C,N,D], bias/avg_activity [C,N], k:

    drive   = x@W_ff + 0.3*(x_ctx@W_ctx) + bias          [B,C,N]
    boosted = drive + log1p(0.05/(avg+1e-6))             (homeostasis*100 = 1.0)
    mask    = boosted >= (k-th largest of boosted per row)
    act     = relu(drive)*mask;  act = act/(act.sum(-1)+1e-8)*k
    pred    = act @ W_pred                                [B,C,D]
    err     = x_input - pred

Sharding: columns C split across 8 NeuronCores (8 each), x replicated.
"""

import os
import numpy as np

P = 128
B, C, D, N = 1024, 64, 256, 512
NCORES = 8
C_LOC = C // NCORES   # 8 columns per core
BT = B // P           # 8 b-tiles per core
KD = D // P           # 2 contraction chunks for the drive matmuls
KN = N // P           # 4 contraction chunks for the prediction matmul

_NEG_FILL = -3.0e38   # match_replace fill; far below any real boosted value

_cache = {}


def ts(i, size):
    return slice(i * size, (i + 1) * size)


def _build(k):
    """Trace + compile the per-core kernel. Returns the compiled Bass module."""
    import concourse.bass as bass  # noqa: F401
    import concourse.bacc as bacc
    import concourse.mybir as mybir
    from concourse import tile
    from concourse.masks import make_identity

    f32 = mybir.dt.float32
    bf16 = mybir.dt.bfloat16
    Alu = mybir.AluOpType
    Act = mybir.ActivationFunctionType

    k = int(k)
    assert 1 <= k <= N
    rounds = (k + 7) // 8          # max8 rounds needed to reach the k-th value
    last_idx = (k - 1) % 8

    nc = bacc.Bacc("TRN2", target_bir_lowering=False, debug=False,
                   num_devices=NCORES)

    xT_d = nc.declare_dram_parameter("xT", [D, B], f32, isOutput=False)
    xcT_d = nc.declare_dram_parameter("xcT", [D, B], f32, isOutput=False)
    x_d = nc.declare_dram_parameter("x", [B, D], f32, isOutput=False)
    wff_d = nc.declare_dram_parameter("wff", [C_LOC, D, N], f32, isOutput=False)
    wctx_d = nc.declare_dram_parameter("wctx", [C_LOC, D, N], f32, isOutput=False)
    wp_d = nc.declare_dram_parameter("wp", [C_LOC, N, D], bf16, isOutput=False)
    btb_d = nc.declare_dram_parameter("btb", [C_LOC, N], f32, isOutput=False)
    bt_d = nc.declare_dram_parameter("bt", [C_LOC, N], f32, isOutput=False)

    act_d = nc.declare_dram_parameter("act", [B, C_LOC, N], f32, isOutput=True)
    pred_d = nc.declare_dram_parameter("pred", [B, C_LOC, D], f32, isOutput=True)
    err_d = nc.declare_dram_parameter("err", [B, C_LOC, D], f32, isOutput=True)

    with tile.TileContext(nc) as tc:
        with (
            tc.tile_pool(name="const", bufs=1) as const,
            tc.tile_pool(name="wpool", bufs=2) as wpool,
            tc.tile_pool(name="work", bufs=3) as work,
            tc.tile_pool(name="psA", bufs=2, space="PSUM") as psA,
            tc.tile_pool(name="psT", bufs=2, space="PSUM") as psT,
            tc.tile_pool(name="psP", bufs=2, space="PSUM") as psP,
            tc.tile_pool(name="psB", bufs=1, space="PSUM") as psB,
        ):
            # ---- resident constants ----
            ident = const.tile([P, P], f32)
            make_identity(nc, ident[:])

            ones_row = const.tile([1, P], f32)
            nc.vector.memset(ones_row[:], 1.0)

            # rows kept on partition 0 so K=1 matmul rhs slices are legal
            btb_s = const.tile([1, C_LOC, N], f32)   # bias + boost rows
            nc.sync.dma_start(btb_s[:], btb_d[:].rearrange("(o c) n -> o c n", o=1))
            bt_s = const.tile([1, C_LOC, N], f32)    # boost-only rows
            nc.sync.dma_start(bt_s[:], bt_d[:].rearrange("(o c) n -> o c n", o=1))

            xT_s = const.tile([P, KD, B], f32)
            nc.sync.dma_start(xT_s[:], xT_d[:].rearrange("(kc p) b -> p kc b", p=P))
            xcT_s = const.tile([P, KD, B], f32)
            nc.sync.dma_start(xcT_s[:], xcT_d[:].rearrange("(kc p) b -> p kc b", p=P))
            x_s = const.tile([P, BT, D], f32)
            nc.sync.dma_start(x_s[:], x_d[:].rearrange("(bt p) d -> p bt d", p=P))

            # bt broadcast tiles [128, N] per local column, built once via a
            # K=1 outer-product matmul (ones_col x bt_row) + ACT copy-out.
            bt_bc = const.tile([P, C_LOC, N], f32)
            for ci in range(C_LOC):
                ps = psB.tile([P, N], f32)
                nc.tensor.matmul(ps[:], lhsT=ones_row[:], rhs=bt_s[:, ci, :],
                                 start=True, stop=True)
                nc.scalar.activation(bt_bc[:, ci, :], ps[:], Act.Copy)

            inv_k = float(1.0 / k)

            for ci in range(C_LOC):
                # ---- per-column weights ----
                wff_t = wpool.tile([P, KD, N], f32)
                nc.sync.dma_start(
                    wff_t[:], wff_d[ci].rearrange("(kc p) n -> p kc n", p=P))
                wctx_t = wpool.tile([P, KD, N], f32)
                nc.sync.dma_start(
                    wctx_t[:], wctx_d[ci].rearrange("(kc p) n -> p kc n", p=P))
                wp_t = wpool.tile([P, KN, D], bf16)
                nc.sync.dma_start(
                    wp_t[:], wp_d[ci].rearrange("(kc p) d -> p kc d", p=P))

                for bi in range(BT):
                    # ---- boosted = x@Wff + 0.3*xc@Wctx + (bias+boost) ----
                    pA = psA.tile([P, N], f32)
                    nc.tensor.matmul(pA[:], lhsT=xT_s[:, 0, ts(bi, P)],
                                     rhs=wff_t[:, 0, :], start=True, stop=False)
                    for kc in range(1, KD):
                        nc.tensor.matmul(pA[:], lhsT=xT_s[:, kc, ts(bi, P)],
                                         rhs=wff_t[:, kc, :], start=False, stop=False)
                    for kc in range(KD):
                        nc.tensor.matmul(pA[:], lhsT=xcT_s[:, kc, ts(bi, P)],
                                         rhs=wctx_t[:, kc, :], start=False, stop=False)
                    nc.tensor.matmul(pA[:], lhsT=ones_row[:],
                                     rhs=btb_s[:, ci, :], start=False, stop=True)

                    boosted = work.tile([P, N], f32)
                    nc.scalar.activation(boosted[:], pA[:], Act.Copy)

                    # drive = boosted - boost  (recovers raw + bias)
                    drive = work.tile([P, N], f32)
                    nc.gpsimd.tensor_sub(drive[:], boosted[:], bt_bc[:, ci, :])
                    relu_d = work.tile([P, N], f32)
                    nc.scalar.activation(relu_d[:], drive[:], Act.Relu)

                    # ---- top-k threshold via 8-wide max + match_replace ----
                    m = work.tile([P, 8], f32, tag="m8")
                    nc.vector.max(m[:], boosted[:])
                    src = boosted
                    for r in range(1, rounds):
                        nxt = work.tile([P, N], f32, tag=f"mr{r}")
                        nc.vector.match_replace(nxt[:], m[:], src[:], _NEG_FILL)
                        m = work.tile([P, 8], f32, tag="m8")
                        nc.vector.max(m[:], nxt[:])
                        src = nxt
                    thr = m[:, last_idx:last_idx + 1]

                    # masked = (boosted >= thr) * relu(drive), rowsum for free
                    masked = work.tile([P, N], f32)
                    rowsum = work.tile([P, 1], f32)
                    nc.vector.scalar_tensor_tensor(
                        masked[:], boosted[:], thr, relu_d[:],
                        op0=Alu.is_ge, op1=Alu.mult, accum_out=rowsum[:])

                    # s = k / (rowsum + 1e-8)
                    t1 = work.tile([P, 1], f32)
                    nc.scalar.activation(t1[:], rowsum[:], Act.Copy,
                                         scale=inv_k, bias=float(1e-8 * inv_k))
                    s = work.tile([P, 1], f32)
                    nc.vector.reciprocal(s[:], t1[:])

                    # final activation out = masked * s (ACT: gpsimd
                    # TENSOR_SCALAR measured 7.5us/op -- never use it)
                    act_out = work.tile([P, N], f32)
                    nc.scalar.activation(act_out[:], masked[:], Act.Copy,
                                         scale=s[:])
                    nc.sync.dma_start(act_d[ts(bi, P), ci, :], act_out[:])

                    # ---- transpose masked -> actT for the pred matmul ----
                    pT = psT.tile([P, N], f32)
                    for j in range(KN):
                        nc.tensor.transpose(pT[:, ts(j, P)], masked[:, ts(j, P)],
                                            ident[:])
                    actT = work.tile([P, N], bf16)
                    nc.scalar.activation(actT[:], pT[:], Act.Copy)

                    # pred = (masked @ Wp) * s
                    pP = psP.tile([P, D], f32)
                    for j in range(KN):
                        nc.tensor.matmul(pP[:], lhsT=actT[:, ts(j, P)],
                                         rhs=wp_t[:, j, :],
                                         start=(j == 0), stop=(j == KN - 1))
                    pred_s = work.tile([P, D], f32)
                    nc.scalar.activation(pred_s[:], pP[:], Act.Copy, scale=s[:])
                    nc.sync.dma_start(pred_d[ts(bi, P), ci, :], pred_s[:])

                    # err = x - pred
                    err_s = work.tile([P, D], f32)
                    nc.gpsimd.tensor_sub(err_s[:], x_s[:, bi, :], pred_s[:])
                    nc.sync.dma_start(err_d[ts(bi, P), ci, :], err_s[:])

    nc.compile()
    return nc


def _get_nc(k):
    k = int(k)
    if k not in _cache:
        _cache[k] = _build(k)
    return _cache[k]


def _bf16():
    import ml_dtypes
    return ml_dtypes.bfloat16


def _prep_inputs(x_input, x_context, W_ff, W_ctx, W_pred, bias, avg_activity):
    """Host-side shard prep: transposes, feedback scale, boost row."""
    x_input = np.ascontiguousarray(x_input, dtype=np.float32)
    xT = np.ascontiguousarray(x_input.T)
    xcT = np.ascontiguousarray((np.asarray(x_context, np.float32) * np.float32(0.3)).T)
    boost = np.log1p(np.float32(0.05) / (np.asarray(avg_activity, np.float32)
                                         + np.float32(1e-6))).astype(np.float32)
    btb = (np.asarray(bias, np.float32) + boost).astype(np.float32)

    in_maps = []
    for core in range(NCORES):
        cs = ts(core, C_LOC)
        in_maps.append({
            "xT": xT,
            "xcT": xcT,
            "x": x_input,
            "wff": np.ascontiguousarray(W_ff[cs], np.float32),
            "wctx": np.ascontiguousarray(W_ctx[cs], np.float32),
            "wp": np.ascontiguousarray(np.asarray(W_pred[cs], np.float32)
                                       .astype(_bf16())),
            "btb": np.ascontiguousarray(btb[cs]),
            "bt": np.ascontiguousarray(boost[cs]),
        })
    return in_maps


def _run(inputs, trace=False):
    from concourse.bass_utils import run_bass_kernel_spmd

    k = int(inputs["k"])
    nc = _get_nc(k)
    in_maps = _prep_inputs(
        inputs["x_input"], inputs["x_context"], inputs["W_ff"],
        inputs["W_ctx"], inputs["W_pred"], inputs["bias"],
        inputs["avg_activity"])
    res = run_bass_kernel_spmd(nc, in_maps, list(range(NCORES)), trace=trace)
    act = np.concatenate([r["act"] for r in res.results], axis=1)
    pred = np.concatenate([r["pred"] for r in res.results], axis=1)
    err = np.concatenate([r["err"] for r in res.results], axis=1)
    return (act, pred, err), res


def kernel(**inputs):
    outs, _ = _run(inputs, trace=False)
    return outs


# revision 18
# speedup vs baseline: 1.8589x; 1.0020x over previous
"""Trainium2 Bass kernel for the Cortex top-k masking module.

Computes, for inputs x_input/x_context [B,D], per-column weights
W_ff/W_ctx [C,D,N], W_pred [C,N,D], bias/avg_activity [C,N], k:

    drive   = x@W_ff + 0.3*(x_ctx@W_ctx) + bias          [B,C,N]
    boosted = drive + log1p(0.05/(avg+1e-6))             (homeostasis*100 = 1.0)
    mask    = boosted >= (k-th largest of boosted per row)
    act     = relu(drive)*mask;  act = act/(act.sum(-1)+1e-8)*k
    pred    = act @ W_pred                                [B,C,D]
    err     = x_input - pred

Sharding: columns C split across 8 NeuronCores (8 each), x replicated.
"""

import os
import numpy as np

P = 128
B, C, D, N = 1024, 64, 256, 512
NCORES = 8
C_LOC = C // NCORES   # 8 columns per core
BT = B // P           # 8 b-tiles per core
KD = D // P           # 2 contraction chunks for the drive matmuls
KN = N // P           # 4 contraction chunks for the prediction matmul

_NEG_FILL = -3.0e38   # match_replace fill; far below any real boosted value

_cache = {}


def ts(i, size):
    return slice(i * size, (i + 1) * size)


def _enable_ldw_opt():
    """Let walrus double-buffer PE weight loads (background weight buffer).

    bass_utils hardcodes --enable-ldw-opt=false; without it every LDWEIGHTS
    serializes against the in-flight MATMUL (~190us of PE time here).
    """
    try:
        import concourse.bass_utils as bu
        if getattr(bu.run_command, "_ldw_patched", False):
            return
        orig = bu.run_command

        def patched(cmd, *a, **kw):
            if isinstance(cmd, list):
                cmd = ["--enable-ldw-opt=true" if c == "--enable-ldw-opt=false"
                       else c for c in cmd]
            return orig(cmd, *a, **kw)

        patched._ldw_patched = True
        bu.run_command = patched
    except Exception:
        pass


_LDW_OPT = False  # walrus rejects ldw-opt at runtime load; keep disabled


def _build(k):
    """Trace + compile the per-core kernel. Returns the compiled Bass module."""
    import concourse.bass as bass  # noqa: F401
    import concourse.bacc as bacc
    import concourse.mybir as mybir
    if _LDW_OPT:
        _enable_ldw_opt()
    from concourse import tile
    from concourse.masks import make_identity

    f32 = mybir.dt.float32
    bf16 = mybir.dt.bfloat16
    Alu = mybir.AluOpType
    Act = mybir.ActivationFunctionType

    k = int(k)
    assert 1 <= k <= N
    rounds = (k + 7) // 8          # max8 rounds needed to reach the k-th value
    last_idx = (k - 1) % 8

    nc = bacc.Bacc("TRN2", target_bir_lowering=False, debug=False,
                   num_devices=NCORES)

    xT_d = nc.declare_dram_parameter("xT", [D, B], f32, isOutput=False)
    xcT_d = nc.declare_dram_parameter("xcT", [D, B], f32, isOutput=False)
    x_d = nc.declare_dram_parameter("x", [B, D], f32, isOutput=False)
    wff_d = nc.declare_dram_parameter("wff", [C_LOC, D, N], f32, isOutput=False)
    wctx_d = nc.declare_dram_parameter("wctx", [C_LOC, D, N], f32, isOutput=False)
    wp_d = nc.declare_dram_parameter("wp", [C_LOC, N, D], bf16, isOutput=False)
    btb_d = nc.declare_dram_parameter("btb", [C_LOC, N], f32, isOutput=False)
    bt_d = nc.declare_dram_parameter("bt", [C_LOC, N], f32, isOutput=False)

    act_d = nc.declare_dram_parameter("act", [B, C_LOC, N], f32, isOutput=True)
    pred_d = nc.declare_dram_parameter("pred", [B, C_LOC, D], f32, isOutput=True)
    err_d = nc.declare_dram_parameter("err", [B, C_LOC, D], f32, isOutput=True)

    with tile.TileContext(nc) as tc:
        with (
            tc.tile_pool(name="const", bufs=1) as const,
            tc.tile_pool(name="wpool", bufs=2) as wpool,
            tc.tile_pool(name="work", bufs=3) as work,
            tc.tile_pool(name="psA", bufs=2, space="PSUM") as psA,
            tc.tile_pool(name="psT", bufs=2, space="PSUM") as psT,
            tc.tile_pool(name="psP", bufs=2, space="PSUM") as psP,
            tc.tile_pool(name="psB", bufs=1, space="PSUM") as psB,
        ):
            # ---- resident constants ----
            ident = const.tile([P, P], f32)
            make_identity(nc, ident[:])

            ones_row = const.tile([1, P], f32)
            nc.vector.memset(ones_row[:], 1.0)

            # rows kept on partition 0 so K=1 matmul rhs slices are legal
            btb_s = const.tile([1, C_LOC, N], f32)   # bias + boost rows
            nc.sync.dma_start(btb_s[:], btb_d[:].rearrange("(o c) n -> o c n", o=1))
            bt_s = const.tile([1, C_LOC, N], f32)    # boost-only rows
            nc.sync.dma_start(bt_s[:], bt_d[:].rearrange("(o c) n -> o c n", o=1))

            xT_s = const.tile([P, KD, B], f32)
            nc.sync.dma_start(xT_s[:], xT_d[:].rearrange("(kc p) b -> p kc b", p=P))
            xcT_s = const.tile([P, KD, B], f32)
            nc.sync.dma_start(xcT_s[:], xcT_d[:].rearrange("(kc p) b -> p kc b", p=P))
            x_s = const.tile([P, BT, D], f32)
            nc.sync.dma_start(x_s[:], x_d[:].rearrange("(bt p) d -> p bt d", p=P))

            # bt broadcast tiles [128, N] per local column, built once via a
            # K=1 outer-product matmul (ones_col x bt_row) + ACT copy-out.
            bt_bc = const.tile([P, C_LOC, N], f32)
            for ci in range(C_LOC):
                ps = psB.tile([P, N], f32)
                nc.tensor.matmul(ps[:], lhsT=ones_row[:], rhs=bt_s[:, ci, :],
                                 start=True, stop=True)
                nc.scalar.activation(bt_bc[:, ci, :], ps[:], Act.Copy)

            inv_k = float(1.0 / k)

            for ci in range(C_LOC):
                # ---- per-column weights ----
                wff_t = wpool.tile([P, KD, N], f32)
                nc.sync.dma_start(
                    wff_t[:], wff_d[ci].rearrange("(kc p) n -> p kc n", p=P))
                wctx_t = wpool.tile([P, KD, N], f32)
                nc.sync.dma_start(
                    wctx_t[:], wctx_d[ci].rearrange("(kc p) n -> p kc n", p=P))
                wp_t = wpool.tile([P, KN, D], bf16)
                nc.sync.dma_start(
                    wp_t[:], wp_d[ci].rearrange("(kc p) d -> p kc d", p=P))

                for bi in range(BT):
                    # ---- boosted = x@Wff + 0.3*xc@Wctx + (bias+boost) ----
                    pA = psA.tile([P, N], f32)
                    nc.tensor.matmul(pA[:], lhsT=xT_s[:, 0, ts(bi, P)],
                                     rhs=wff_t[:, 0, :], start=True, stop=False)
                    for kc in range(1, KD):
                        nc.tensor.matmul(pA[:], lhsT=xT_s[:, kc, ts(bi, P)],
                                         rhs=wff_t[:, kc, :], start=False, stop=False)
                    for kc in range(KD):
                        nc.tensor.matmul(pA[:], lhsT=xcT_s[:, kc, ts(bi, P)],
                                         rhs=wctx_t[:, kc, :], start=False, stop=False)
                    nc.tensor.matmul(pA[:], lhsT=ones_row[:],
                                     rhs=btb_s[:, ci, :], start=False, stop=True)

                    boosted = work.tile([P, N], f32)
                    nc.scalar.activation(boosted[:], pA[:], Act.Copy)

                    # drive = boosted - boost  (recovers raw + bias)
                    drive = work.tile([P, N], f32)
                    nc.gpsimd.tensor_sub(drive[:], boosted[:], bt_bc[:, ci, :])
                    relu_d = work.tile([P, N], f32)
                    nc.scalar.activation(relu_d[:], drive[:], Act.Relu)

                    # ---- top-k threshold via 8-wide max + match_replace ----
                    m = work.tile([P, 8], f32, tag="m8")
                    nc.vector.max(m[:], boosted[:])
                    src = boosted
                    for r in range(1, rounds):
                        nxt = work.tile([P, N], f32, tag=f"mr{r}")
                        nc.vector.match_replace(nxt[:], m[:], src[:], _NEG_FILL)
                        m = work.tile([P, 8], f32, tag="m8")
                        nc.vector.max(m[:], nxt[:])
                        src = nxt
                    thr = m[:, last_idx:last_idx + 1]

                    # masked = (boosted >= thr) * relu(drive), rowsum for free
                    masked = work.tile([P, N], f32)
                    rowsum = work.tile([P, 1], f32)
                    nc.vector.scalar_tensor_tensor(
                        masked[:], boosted[:], thr, relu_d[:],
                        op0=Alu.is_ge, op1=Alu.mult, accum_out=rowsum[:])

                    # s = k / (rowsum + 1e-8)
                    t1 = work.tile([P, 1], f32)
                    nc.scalar.activation(t1[:], rowsum[:], Act.Copy,
                                         scale=inv_k, bias=float(1e-8 * inv_k))
                    s = work.tile([P, 1], f32)
                    nc.vector.reciprocal(s[:], t1[:])

                    # final activation out = masked * s (ACT: gpsimd
                    # TENSOR_SCALAR measured 7.5us/op -- never use it)
                    act_out = work.tile([P, N], f32)
                    nc.scalar.activation(act_out[:], masked[:], Act.Copy,
                                         scale=s[:])
                    nc.sync.dma_start(act_d[ts(bi, P), ci, :], act_out[:])

                    # ---- transpose masked -> actT for the pred matmul ----
                    pT = psT.tile([P, N], f32)
                    for j in range(KN):
                        nc.tensor.transpose(pT[:, ts(j, P)], masked[:, ts(j, P)],
                                            ident[:])
                    actT = work.tile([P, N], bf16)
                    nc.scalar.activation(actT[:], pT[:], Act.Copy)

                    # pred = (masked @ Wp) * s
                    pP = psP.tile([P, D], f32)
                    for j in range(KN):
                        nc.tensor.matmul(pP[:], lhsT=actT[:, ts(j, P)],
                                         rhs=wp_t[:, j, :],
                                         start=(j == 0), stop=(j == KN - 1))
                    pred_s = work.tile([P, D], f32)
                    nc.scalar.activation(pred_s[:], pP[:], Act.Copy, scale=s[:])
                    nc.sync.dma_start(pred_d[ts(bi, P), ci, :], pred_s[:])

                    # err = x - pred
                    err_s = work.tile([P, D], f32)
                    nc.gpsimd.tensor_sub(err_s[:], x_s[:, bi, :], pred_s[:])
                    nc.sync.dma_start(err_d[ts(bi, P), ci, :], err_s[:])

    nc.compile()
    return nc


def _get_nc(k):
    k = int(k)
    if k not in _cache:
        _cache[k] = _build(k)
    return _cache[k]


def _bf16():
    import ml_dtypes
    return ml_dtypes.bfloat16


def _prep_inputs(x_input, x_context, W_ff, W_ctx, W_pred, bias, avg_activity):
    """Host-side shard prep: transposes, feedback scale, boost row."""
    x_input = np.ascontiguousarray(x_input, dtype=np.float32)
    xT = np.ascontiguousarray(x_input.T)
    xcT = np.ascontiguousarray((np.asarray(x_context, np.float32) * np.float32(0.3)).T)
    boost = np.log1p(np.float32(0.05) / (np.asarray(avg_activity, np.float32)
                                         + np.float32(1e-6))).astype(np.float32)
    btb = (np.asarray(bias, np.float32) + boost).astype(np.float32)

    in_maps = []
    for core in range(NCORES):
        cs = ts(core, C_LOC)
        in_maps.append({
            "xT": xT,
            "xcT": xcT,
            "x": x_input,
            "wff": np.ascontiguousarray(W_ff[cs], np.float32),
            "wctx": np.ascontiguousarray(W_ctx[cs], np.float32),
            "wp": np.ascontiguousarray(np.asarray(W_pred[cs], np.float32)
                                       .astype(_bf16())),
            "btb": np.ascontiguousarray(btb[cs]),
            "bt": np.ascontiguousarray(boost[cs]),
        })
    return in_maps


def _run(inputs, trace=False):
    from concourse.bass_utils import run_bass_kernel_spmd

    k = int(inputs["k"])
    nc = _get_nc(k)
    in_maps = _prep_inputs(
        inputs["x_input"], inputs["x_context"], inputs["W_ff"],
        inputs["W_ctx"], inputs["W_pred"], inputs["bias"],
        inputs["avg_activity"])
    res = run_bass_kernel_spmd(nc, in_maps, list(range(NCORES)), trace=trace)
    act = np.concatenate([r["act"] for r in res.results], axis=1)
    pred = np.concatenate([r["pred"] for r in res.results], axis=1)
    err = np.concatenate([r["err"] for r in res.results], axis=1)
    return (act, pred, err), res


def kernel(**inputs):
    outs, _ = _run(inputs, trace=False)
    return outs


# revision 20
# speedup vs baseline: 2.1984x; 1.1826x over previous
"""Trainium2 Bass kernel for the Cortex top-k masking module.

Computes, for inputs x_input/x_context [B,D], per-column weights
W_ff/W_ctx [C,D,N], W_pred [C,N,D], bias/avg_activity [C,N], k:

    drive   = x@W_ff + 0.3*(x_ctx@W_ctx) + bias          [B,C,N]
    boosted = drive + log1p(0.05/(avg+1e-6))             (homeostasis*100 = 1.0)
    mask    = boosted >= (k-th largest of boosted per row)
    act     = relu(drive)*mask;  act = act/(act.sum(-1)+1e-8)*k
    pred    = act @ W_pred                                [B,C,D]
    err     = x_input - pred

Sharding: columns C split across 8 NeuronCores (8 each), x replicated.
"""

import os
import numpy as np

P = 128
B, C, D, N = 1024, 64, 256, 512
NCORES = 8
C_LOC = C // NCORES   # 8 columns per core
BT = B // P           # 8 b-tiles per core
KD = D // P           # 2 contraction chunks for the drive matmuls
KN = N // P           # 4 contraction chunks for the prediction matmul

_NEG_FILL = -3.0e38   # match_replace fill; far below any real boosted value

_cache = {}


def ts(i, size):
    return slice(i * size, (i + 1) * size)


def _enable_ldw_opt():
    """Let walrus double-buffer PE weight loads (background weight buffer).

    bass_utils hardcodes --enable-ldw-opt=false; without it every LDWEIGHTS
    serializes against the in-flight MATMUL (~190us of PE time here).
    """
    try:
        import concourse.bass_utils as bu
        if getattr(bu.run_command, "_ldw_patched", False):
            return
        orig = bu.run_command

        def patched(cmd, *a, **kw):
            if isinstance(cmd, list):
                cmd = ["--enable-ldw-opt=true" if c == "--enable-ldw-opt=false"
                       else c for c in cmd]
            return orig(cmd, *a, **kw)

        patched._ldw_patched = True
        bu.run_command = patched
    except Exception:
        pass


_LDW_OPT = False  # walrus rejects ldw-opt at runtime load; keep disabled


def _build(k):
    """Trace + compile the per-core kernel. Returns the compiled Bass module."""
    import concourse.bass as bass  # noqa: F401
    import concourse.bacc as bacc
    import concourse.mybir as mybir
    if _LDW_OPT:
        _enable_ldw_opt()
    from concourse import tile
    from concourse.masks import make_identity

    f32 = mybir.dt.float32
    bf16 = mybir.dt.bfloat16
    Alu = mybir.AluOpType
    Act = mybir.ActivationFunctionType

    k = int(k)
    assert 1 <= k <= N
    rounds = (k + 7) // 8          # max8 rounds needed to reach the k-th value
    last_idx = (k - 1) % 8

    nc = bacc.Bacc("TRN2", target_bir_lowering=False, debug=False,
                   num_devices=NCORES)

    xT_d = nc.declare_dram_parameter("xT", [D, B], f32, isOutput=False)
    xcT_d = nc.declare_dram_parameter("xcT", [D, B], f32, isOutput=False)
    x_d = nc.declare_dram_parameter("x", [B, D], f32, isOutput=False)
    wff_d = nc.declare_dram_parameter("wff", [C_LOC, D, N], f32, isOutput=False)
    wctx_d = nc.declare_dram_parameter("wctx", [C_LOC, D, N], f32, isOutput=False)
    wp_d = nc.declare_dram_parameter("wp", [C_LOC, N, D], bf16, isOutput=False)
    btb_d = nc.declare_dram_parameter("btb", [C_LOC, N], f32, isOutput=False)
    bt_d = nc.declare_dram_parameter("bt", [C_LOC, N], f32, isOutput=False)

    act_d = nc.declare_dram_parameter("act", [B, C_LOC, N], f32, isOutput=True)
    pred_d = nc.declare_dram_parameter("pred", [B, C_LOC, D], f32, isOutput=True)
    err_d = nc.declare_dram_parameter("err", [B, C_LOC, D], f32, isOutput=True)

    with tile.TileContext(nc) as tc:
        with (
            tc.tile_pool(name="const", bufs=1) as const,
            tc.tile_pool(name="wpool", bufs=2) as wpool,
            tc.tile_pool(name="work", bufs=3) as work,
            tc.tile_pool(name="psA", bufs=2, space="PSUM") as psA,
            tc.tile_pool(name="psT", bufs=2, space="PSUM") as psT,
            tc.tile_pool(name="psP", bufs=2, space="PSUM") as psP,
            tc.tile_pool(name="psB", bufs=1, space="PSUM") as psB,
        ):
            # ---- resident constants ----
            ident = const.tile([P, P], f32)
            make_identity(nc, ident[:])

            ones_row = const.tile([1, P], f32)
            nc.vector.memset(ones_row[:], 1.0)

            # rows kept on partition 0 so K=1 matmul rhs slices are legal
            btb_s = const.tile([1, C_LOC, N], f32)   # bias + boost rows
            nc.sync.dma_start(btb_s[:], btb_d[:].rearrange("(o c) n -> o c n", o=1))
            bt_s = const.tile([1, C_LOC, N], f32)    # boost-only rows
            nc.sync.dma_start(bt_s[:], bt_d[:].rearrange("(o c) n -> o c n", o=1))

            xT_s = const.tile([P, KD, B], f32)
            nc.sync.dma_start(xT_s[:], xT_d[:].rearrange("(kc p) b -> p kc b", p=P))
            xcT_s = const.tile([P, KD, B], f32)
            nc.sync.dma_start(xcT_s[:], xcT_d[:].rearrange("(kc p) b -> p kc b", p=P))
            x_s = const.tile([P, BT, D], f32)
            nc.sync.dma_start(x_s[:], x_d[:].rearrange("(bt p) d -> p bt d", p=P))

            # bt broadcast tiles [128, N] per local column, built once via a
            # K=1 outer-product matmul (ones_col x bt_row) + ACT copy-out.
            bt_bc = const.tile([P, C_LOC, N], f32)
            btb_bc = const.tile([P, C_LOC, N], f32)
            for ci in range(C_LOC):
                ps = psB.tile([P, N], f32)
                nc.tensor.matmul(ps[:], lhsT=ones_row[:], rhs=bt_s[:, ci, :],
                                 start=True, stop=True)
                nc.scalar.activation(bt_bc[:, ci, :], ps[:], Act.Copy)
                ps2 = psB.tile([P, N], f32)
                nc.tensor.matmul(ps2[:], lhsT=ones_row[:], rhs=btb_s[:, ci, :],
                                 start=True, stop=True)
                nc.scalar.activation(btb_bc[:, ci, :], ps2[:], Act.Copy)

            inv_k = float(1.0 / k)

            for ci in range(C_LOC):
                # ---- per-column weights ----
                wff_t = wpool.tile([P, KD, N], f32)
                nc.sync.dma_start(
                    wff_t[:], wff_d[ci].rearrange("(kc p) n -> p kc n", p=P))
                wctx_t = wpool.tile([P, KD, N], f32)
                nc.sync.dma_start(
                    wctx_t[:], wctx_d[ci].rearrange("(kc p) n -> p kc n", p=P))
                wp_t = wpool.tile([P, KN, D], bf16)
                nc.sync.dma_start(
                    wp_t[:], wp_d[ci].rearrange("(kc p) d -> p kc d", p=P))

                for bi in range(BT):
                    # ---- boosted = x@Wff + 0.3*xc@Wctx + (bias+boost) ----
                    pA = psA.tile([P, N], f32)
                    nc.tensor.matmul(pA[:], lhsT=xT_s[:, 0, ts(bi, P)],
                                     rhs=wff_t[:, 0, :], start=True, stop=False)
                    for kc in range(1, KD):
                        nc.tensor.matmul(pA[:], lhsT=xT_s[:, kc, ts(bi, P)],
                                         rhs=wff_t[:, kc, :], start=False, stop=False)
                    for kc in range(KD):
                        nc.tensor.matmul(pA[:], lhsT=xcT_s[:, kc, ts(bi, P)],
                                         rhs=wctx_t[:, kc, :], start=False,
                                         stop=(kc == KD - 1))

                    # boosted = pA + (bias+boost): DVE add straight out of
                    # PSUM -- frees both the PE btb outer and the ACT copy.
                    boosted = work.tile([P, N], f32)
                    nc.vector.tensor_add(boosted[:], pA[:], btb_bc[:, ci, :])

                    # drive = boosted - boost  (recovers raw + bias)
                    drive = work.tile([P, N], f32)
                    nc.gpsimd.tensor_sub(drive[:], boosted[:], bt_bc[:, ci, :])
                    relu_d = work.tile([P, N], f32)
                    nc.scalar.activation(relu_d[:], drive[:], Act.Relu)

                    # ---- top-k threshold via 8-wide max + match_replace ----
                    m = work.tile([P, 8], f32, tag="m8")
                    nc.vector.max(m[:], boosted[:])
                    src = boosted
                    for r in range(1, rounds):
                        nxt = work.tile([P, N], f32, tag=f"mr{r}")
                        nc.vector.match_replace(nxt[:], m[:], src[:], _NEG_FILL)
                        m = work.tile([P, 8], f32, tag="m8")
                        nc.vector.max(m[:], nxt[:])
                        src = nxt
                    thr = m[:, last_idx:last_idx + 1]

                    # masked = (boosted >= thr) * relu(drive), rowsum for free
                    masked = work.tile([P, N], f32)
                    rowsum = work.tile([P, 1], f32)
                    nc.vector.scalar_tensor_tensor(
                        masked[:], boosted[:], thr, relu_d[:],
                        op0=Alu.is_ge, op1=Alu.mult, accum_out=rowsum[:])

                    # s = k / (rowsum + 1e-8)
                    t1 = work.tile([P, 1], f32)
                    nc.scalar.activation(t1[:], rowsum[:], Act.Copy,
                                         scale=inv_k, bias=float(1e-8 * inv_k))
                    s = work.tile([P, 1], f32)
                    nc.vector.reciprocal(s[:], t1[:])

                    # final activation out = masked * s (ACT: gpsimd
                    # TENSOR_SCALAR measured 7.5us/op -- never use it)
                    act_out = work.tile([P, N], f32)
                    nc.scalar.activation(act_out[:], masked[:], Act.Copy,
                                         scale=s[:])
                    nc.sync.dma_start(act_d[ts(bi, P), ci, :], act_out[:])

                    # ---- transpose masked -> actT for the pred matmul ----
                    pT = psT.tile([P, N], f32)
                    for j in range(KN):
                        nc.tensor.transpose(pT[:, ts(j, P)], masked[:, ts(j, P)],
                                            ident[:])
                    actT = work.tile([P, N], bf16)
                    nc.scalar.activation(actT[:], pT[:], Act.Copy)

                    # pred = (masked @ Wp) * s
                    pP = psP.tile([P, D], f32)
                    for j in range(KN):
                        nc.tensor.matmul(pP[:], lhsT=actT[:, ts(j, P)],
                                         rhs=wp_t[:, j, :],
                                         start=(j == 0), stop=(j == KN - 1))
                    pred_s = work.tile([P, D], f32)
                    nc.scalar.activation(pred_s[:], pP[:], Act.Copy, scale=s[:])
                    nc.sync.dma_start(pred_d[ts(bi, P), ci, :], pred_s[:])

                    # err = x - pred
                    err_s = work.tile([P, D], f32)
                    nc.gpsimd.tensor_sub(err_s[:], x_s[:, bi, :], pred_s[:])
                    nc.sync.dma_start(err_d[ts(bi, P), ci, :], err_s[:])

    nc.compile()
    return nc


def _get_nc(k):
    k = int(k)
    if k not in _cache:
        _cache[k] = _build(k)
    return _cache[k]


def _bf16():
    import ml_dtypes
    return ml_dtypes.bfloat16


def _prep_inputs(x_input, x_context, W_ff, W_ctx, W_pred, bias, avg_activity):
    """Host-side shard prep: transposes, feedback scale, boost row."""
    x_input = np.ascontiguousarray(x_input, dtype=np.float32)
    xT = np.ascontiguousarray(x_input.T)
    xcT = np.ascontiguousarray((np.asarray(x_context, np.float32) * np.float32(0.3)).T)
    boost = np.log1p(np.float32(0.05) / (np.asarray(avg_activity, np.float32)
                                         + np.float32(1e-6))).astype(np.float32)
    btb = (np.asarray(bias, np.float32) + boost).astype(np.float32)

    in_maps = []
    for core in range(NCORES):
        cs = ts(core, C_LOC)
        in_maps.append({
            "xT": xT,
            "xcT": xcT,
            "x": x_input,
            "wff": np.ascontiguousarray(W_ff[cs], np.float32),
            "wctx": np.ascontiguousarray(W_ctx[cs], np.float32),
            "wp": np.ascontiguousarray(np.asarray(W_pred[cs], np.float32)
                                       .astype(_bf16())),
            "btb": np.ascontiguousarray(btb[cs]),
            "bt": np.ascontiguousarray(boost[cs]),
        })
    return in_maps


def _run(inputs, trace=False):
    from concourse.bass_utils import run_bass_kernel_spmd

    k = int(inputs["k"])
    nc = _get_nc(k)
    in_maps = _prep_inputs(
        inputs["x_input"], inputs["x_context"], inputs["W_ff"],
        inputs["W_ctx"], inputs["W_pred"], inputs["bias"],
        inputs["avg_activity"])
    res = run_bass_kernel_spmd(nc, in_maps, list(range(NCORES)), trace=trace)
    act = np.concatenate([r["act"] for r in res.results], axis=1)
    pred = np.concatenate([r["pred"] for r in res.results], axis=1)
    err = np.concatenate([r["err"] for r in res.results], axis=1)
    return (act, pred, err), res


def kernel(**inputs):
    outs, _ = _run(inputs, trace=False)
    return outs
